# revision 7
# baseline (speedup 1.0000x reference)
"""CameraAwareMemory loss kernel for 8 Trainium2 NeuronCores (v2).

Strategy: shard the P=32768 proxy bank over 8 cores (4096 proxies each,
natural order; camera of proxy p is p % 8).  Each core computes the single
GEMM that matters, sims16 = (feat + r*mem[prx]) @ (16*mem_shard)^T, with one
fp8e4 DoubleRow matmul per [128 x 1024] PSUM tile (K=256 folded into the PE's
double-row mode), then *reduces* each tile on the way out:

  - DVE tiles: one strided tensor_reduce computes camera-aligned max-groups
    of 8 (columns {c + 64m}) straight from PSUM, bf16 out.
  - Act/Pool tiles: the scalar engine evacuates exp(BETA * v) to bf16 SBUF
    (same cost as a copy) and GPSIMD sum-folds it by 4 (columns {c + 256j}).
    log(mass)/BETA ranks groups within ln(4)/BETA of the true group max,
    which is all selection needs.

Only ~0.4 KiB/partition of group summaries leave each core.  The host picks
the top-T groups per (row, camera) across cores, expands them to member
proxies, recomputes exact fp32/f64 scores at those candidates, and assembles
the exact intra / cross / online losses (with TEMP=0.05 every reference
logsumexp is dominated by its top handful of terms, so candidate coverage is
all that matters; positives and the target proxy are always included
exactly).
"""

import sys

import numpy as np

sys.path.insert(0, "/opt/trn_rl_repo")

# ---- problem constants (hardcoded per spec) ----
P = 32768
D = 256
C = 8
B = 256
TEMP = 0.05
BG_KNN = 50
POSK = 3
BAL_W = 0.15
RATIO = (1.0 - BAL_W) / BAL_W        # sims' = score + RATIO*q (same order as sims)
INV_TEMP = 1.0 / TEMP                # 20.0
NCORES = 8
PSH = P // NCORES                    # 4096 proxies per core
MEMSCALE = 16.0                      # mem is shipped as fp8(16*mem)
BETA = 0.5                           # exp-pool sharpness (on 16*sims' units)

NTILES = 4                           # column tiles of 1024 per row-tile
TILEW = PSH // NTILES                # 1024
# lane per (rt, i) tile: 'dve' (max fold-8) or 'ap' (exp fold-4)
LANES = ["dve", "ap", "dve", "ap"]
W_LANE = {"dve": TILEW // 8, "ap": TILEW // 4}      # out cols per tile
RT_W = sum(W_LANE[l] for l in LANES)                # 768
OUT_W = 2 * RT_W                                     # 1536
GROUP = {"dve": 8, "ap": 4}
N_IN_CHUNKS = 4                       # input DMA chunks (first includes lhsT)

_CACHE = {}


def _build_bass():
    import concourse.bacc as bacc
    import concourse.mybir as mybir
    import concourse.tile as tile
    from contextlib import ExitStack

    f32 = mybir.dt.float32
    f8 = mybir.dt.float8e4
    bf16 = mybir.dt.bfloat16
    AF = mybir.ActivationFunctionType
    ALU = mybir.AluOpType

    nc = bacc.Bacc("TRN2", target_bir_lowering=False, debug=False)

    PK = 256 + PSH                     # lhsT cols + mem cols per k-tile
    pack_d = nc.dram_tensor("pack", [128, 2, PK], f8, kind="ExternalInput")
    out_d = nc.dram_tensor("out", [128, OUT_W], bf16, kind="ExternalOutput")

    with tile.TileContext(nc) as tc, ExitStack() as ctx:
        consts = ctx.enter_context(tc.tile_pool(name="consts", bufs=1))
        psum = ctx.enter_context(tc.tile_pool(name="psum", bufs=3, space="PSUM"))
        psum_warm = ctx.enter_context(
            tc.tile_pool(name="psumw", bufs=1, space="PSUM"))
        rawp = ctx.enter_context(tc.tile_pool(name="rawp", bufs=2))
        t1p = ctx.enter_context(tc.tile_pool(name="t1p", bufs=2))
        stage = ctx.enter_context(tc.tile_pool(name="stage", bufs=1))

        pack_sb = consts.tile([128, 2, PK], f8, tag="pack")
        # input chunks: chunk 0 carries lhsT + mem tile 0 so compute starts
        # early; the rest arrive one mem tile at a time.
        bounds = [0, 256 + TILEW]
        for i in range(1, NTILES):
            bounds.append(256 + (i + 1) * TILEW)
        for g in range(len(bounds) - 1):
            lo, hi = bounds[g], bounds[g + 1]
            nc.sync.dma_start(out=pack_sb[:, :, lo:hi], in_=pack_d[:, :, lo:hi])

        def lhsT(rt):
            return pack_sb[:, :, rt * 128:(rt + 1) * 128]

        # PE warm-up while chunk 0 streams in: DoubleRow matmuls on the lhsT
        # region (never-read results) keep the PE clock out of its lowest
        # p-state for the real matmuls.
        warm_ps = psum_warm.tile([128, 128], f32, tag="warmps")
        for _ in range(4):
            nc.tensor.matmul(
                warm_ps[:], lhsT=lhsT(0), rhs=lhsT(1),
                start=True, stop=True,
                perf_mode=mybir.MatmulPerfMode.DoubleRow,
            )

        stage_t = [
            stage.tile([128, RT_W], bf16, tag=f"stage{rt}", name=f"stage_{rt}")
            for rt in range(2)
        ]

        for i in range(NTILES):
            for rt in range(2):
                ps = psum.tile([128, TILEW], f32, tag="ps", name=f"ps_{i}_{rt}")
                # one matmul per PSUM bank (an output AP may not span banks)
                for h in range(TILEW // 512):
                    lo = 256 + i * TILEW + h * 512
                    nc.tensor.matmul(
                        ps[:, h * 512:(h + 1) * 512], lhsT=lhsT(rt),
                        rhs=pack_sb[:, :, lo:lo + 512],
                        start=True, stop=True,
                        perf_mode=mybir.MatmulPerfMode.DoubleRow,
                    )
                off = sum(W_LANE[LANES[j]] for j in range(i))
                w = W_LANE[LANES[i]]
                dst = stage_t[rt][:, off:off + w]
                if LANES[i] == "dve":
                    # camera-aligned max fold-8: out[c] = max_m ps[c + (w)m]
                    nc.vector.tensor_reduce(
                        out=dst,
                        in_=ps[:].rearrange("p (m rest) -> p rest m", m=8),
                        axis=mybir.AxisListType.X, op=ALU.max)
                else:
                    raw = rawp.tile([128, TILEW], bf16, tag="raw",
                                    name=f"raw_{i}_{rt}")
                    nc.scalar.activation(raw[:], ps[:], AF.Exp, scale=BETA)
                    t1 = t1p.tile([128, TILEW // 2], bf16, tag="t1",
                                  name=f"t1_{i}_{rt}")
                    nc.gpsimd.tensor_tensor(
                        out=t1[:], in0=raw[:, 0:TILEW // 2],
                        in1=raw[:, TILEW // 2:TILEW], op=ALU.add)
                    nc.gpsimd.tensor_tensor(
                        out=dst, in0=t1[:, 0:TILEW // 4],
                        in1=t1[:, TILEW // 4:TILEW // 2], op=ALU.add)

        for rt in range(2):
            nc.sync.dma_start(
                out=out_d[:, rt * RT_W:(rt + 1) * RT_W], in_=stage_t[rt][:])

    nc.compile()
    return nc


def _get_nc():
    if "nc" not in _CACHE:
        _CACHE["nc"] = _build_bass()
    return _CACHE["nc"]


def _run_device(in_maps, trace=False):
    from concourse.bass_utils import run_bass_kernel_spmd

    nc = _get_nc()
    res = run_bass_kernel_spmd(
        nc, in_maps, core_ids=list(range(NCORES)), trace=trace
    )
    return res


def _group_tables():
    """Static per-core tables mapping out columns -> candidate member local
    proxy ids [OUT_W, 8] (pad -1) and member count; plus each column's camera.
    Local pid = offset within the core's 4096-proxy shard."""
    if "tables" in _CACHE:
        return _CACHE["tables"]
    members = np.full((OUT_W, 8), -1, np.int64)
    for rt in range(2):
        base = rt * RT_W
        off = 0
        for i in range(NTILES):
            lane = LANES[i]
            w = W_LANE[lane]
            cols = np.arange(w)
            if lane == "dve":
                # groups {c + (w)m, m=0..7} within tile i
                mem = i * TILEW + cols[:, None] + w * np.arange(8)[None, :]
                members[base + off + cols] = mem
            else:
                mem4 = i * TILEW + cols[:, None] + w * np.arange(4)[None, :]
                members[base + off + cols, 0:4] = mem4
            off += w
    cam_of_col = np.where(members[:, 0] >= 0, members[:, 0] % C, -1)
    is_ap = np.zeros(OUT_W, bool)
    for rt in range(2):
        base = rt * RT_W
        off = 0
        for i in range(NTILES):
            w = W_LANE[LANES[i]]
            if LANES[i] == "ap":
                is_ap[base + off:base + off + w] = True
            off += w
    _CACHE["tables"] = (members, cam_of_col, is_ap)
    return _CACHE["tables"]


def _logsumexp(x, axis=-1):
    m = np.max(x, axis=axis, keepdims=True)
    m = np.where(np.isfinite(m), m, 0.0)
    return np.squeeze(m, axis) + np.log(
        np.sum(np.exp(x - m), axis=axis))


TOPG = 20          # groups kept per (row, camera)


def kernel(features, targets, cams, epoch, global_memory, all_pseudo_label,
           all_proxy_label, cam_proxies, label_proxies, _want_trace=False):
    import ml_dtypes

    feat = np.ascontiguousarray(np.asarray(features), dtype=np.float32)
    mem = np.ascontiguousarray(np.asarray(global_memory), dtype=np.float32)
    targets = np.asarray(targets).astype(np.int64)
    cams_h = np.asarray(cams).astype(np.int64)
    apl = np.asarray(all_proxy_label).astype(np.int64)
    apsl = np.asarray(all_pseudo_label).astype(np.int64)
    lab_prox = np.asarray(label_proxies).astype(np.int64)

    prx = apl[targets]                      # [B] target proxy
    pseudo_y = apsl[targets]                # [B]
    pos_cols = lab_prox[pseudo_y]           # [B, C] positive proxies (cross)
    memprx = mem[prx]                       # [B, D]

    # ---- device inputs: fp8 pack per core ----
    f8dt = ml_dtypes.float8_e4m3
    lhs = (feat + np.float32(RATIO) * memprx).astype(f8dt)      # [B, D]
    memq = (mem.T * np.float32(MEMSCALE)).astype(f8dt)          # [D, P]
    lhsT = lhs.T                                                # [D, B] f8

    in_maps = []
    for k in range(NCORES):
        pack = np.empty((128, 2, 256 + PSH), f8dt)
        for i in range(2):
            pack[:, i, 0:256] = lhsT[i * 128:(i + 1) * 128, :]
            pack[:, i, 256:] = memq[i * 128:(i + 1) * 128,
                                    k * PSH:(k + 1) * PSH]
        in_maps.append({"pack": np.ascontiguousarray(pack)})

    res = _run_device(in_maps, trace=_want_trace)
    results = res.results
    if _want_trace:
        _CACHE["last_exec_time_ns"] = res.exec_time_ns

    members, cam_of_col, is_ap = _group_tables()

    # ---- decode group scores (units of 16*sims'; higher = hotter group) ----
    # out rows are partition rows; batch row = rt*128 + r, rt from col block.
    raw = np.stack([r["out"].astype(np.float32) for r in results])  # [K,128,OUT_W]
    scores = np.empty((NCORES, 2, 128, RT_W), np.float32)
    for rt in range(2):
        scores[:, rt] = raw[:, :, rt * RT_W:(rt + 1) * RT_W]
    # -> [B, K, RT_W] with batch row = rt*128 + r (rt-major)
    scores = np.transpose(scores, (1, 2, 0, 3)).reshape(B, NCORES, RT_W)
    # columns within one rt block
    mem_rt = members[0:RT_W]
    cam_rt = cam_of_col[0:RT_W]
    ap_rt = is_ap[0:RT_W]
    # exp-mass columns -> log scale
    apcols = np.where(ap_rt)[0]
    sc_ap = scores[:, :, apcols]
    np.maximum(sc_ap, 1e-30, out=sc_ap)
    scores[:, :, apcols] = np.log(sc_ap) / np.float32(BETA)

    # ---- per (row, camera) top-TOPG groups across cores ----
    order = np.argsort(cam_rt, kind="stable")
    cam_sorted = cam_rt[order]
    percam = RT_W // C
    assert np.all(cam_sorted.reshape(C, percam) == np.arange(C)[:, None])
    sc_bycam = scores[:, :, order].transpose(0, 2, 1).reshape(B, C, percam, NCORES)
    # -> [B, C, NCORES*percam]
    sc_bycam = sc_bycam.transpose(0, 1, 3, 2).reshape(B, C, NCORES * percam)
    topi = np.argpartition(-sc_bycam, TOPG, axis=2)[:, :, :TOPG]  # [B,C,T]
    core_i = topi // percam
    col_i = order[np.mod(topi, percam) + np.arange(C)[None, :, None] * percam]
    # member local pids -> global pids
    memb = mem_rt[col_i]                                  # [B,C,T,8] local
    valid = memb >= 0
    pid = np.where(valid, memb + core_i[..., None] * PSH, -1)  # [B,C,T,8]

    cand = pid.reshape(B, -1)                             # [B, C*T*8]
    vmask = cand >= 0
    cand_safe = np.where(vmask, cand, 0)

    # ---- exact recompute at candidates ----
    mg = mem[cand_safe]                                   # [B, N, D]
    s_cand = np.einsum("bd,bnd->bn", feat, mg).astype(np.float64)
    q_cand = np.einsum("bd,bnd->bn", memprx, mg).astype(np.float64)
    x_cand = INV_TEMP * s_cand
    simsp = s_cand + RATIO * q_cand
    NEG = -1e30
    x_cand = np.where(vmask, x_cand, NEG)
    simsp = np.where(vmask, simsp, NEG)

    rows = np.arange(B)
    x_prx = INV_TEMP * np.einsum("bd,bd->b", feat.astype(np.float64),
                                 memprx.astype(np.float64))
    x_pos = INV_TEMP * np.einsum("bd,bkd->bk", feat.astype(np.float64),
                                 mem[pos_cols].astype(np.float64))

    # ---- intra: lse over own camera's proxies minus x_prx ----
    NC_CAM = TOPG * 8
    cand_c = cand.reshape(B, C, NC_CAM)
    x_c = x_cand.reshape(B, C, NC_CAM)
    simsp_c = simsp.reshape(B, C, NC_CAM)
    own = cams_h                                         # [B]
    x_own = x_c[rows, own]                               # [B, NC_CAM]
    cand_own = cand_c[rows, own]
    x_own = np.where(cand_own == prx[:, None], NEG, x_own)
    present = (prx % C) == cams_h
    lse_cam = _logsumexp(np.concatenate([x_own, x_prx[:, None]], axis=1))
    intra = np.where(present, lse_cam - x_prx, 0.0)

    # ---- cross: lse over (candidates \ positives) + positives ----
    x_cr = np.where((cand[:, :, None] == pos_cols[:, None, :]).any(axis=2),
                    NEG, x_cand)
    lse_full = _logsumexp(np.concatenate([x_cr, x_pos], axis=1))
    cross = lse_full - x_pos.mean(axis=1)

    # ---- online ----
    arg = np.argmax(simsp_c, axis=2)                      # [B, C]
    tops_val = np.take_along_axis(simsp_c, arg[:, :, None], axis=2)[:, :, 0]
    tops_pid = np.take_along_axis(cand_c, arg[:, :, None], axis=2)[:, :, 0]
    x_top_cam = np.take_along_axis(x_c, arg[:, :, None], axis=2)[:, :, 0]
    sel_c = np.argsort(-tops_val, axis=1)[:, :POSK]       # [B, 3] cameras
    chosen_pid = np.take_along_axis(tops_pid, sel_c, axis=1)   # [B, 3]
    x_tops = np.take_along_axis(x_top_cam, sel_c, axis=1)      # [B, 3]

    is_chosen = (cand[:, :, None] == chosen_pid[:, None, :]).any(axis=2)
    simsp_m = np.where(is_chosen, NEG, simsp)
    sel_idx = np.argpartition(-simsp_m, BG_KNN, axis=1)[:, :BG_KNN]
    x_sel = np.take_along_axis(x_cand, sel_idx, axis=1)
    # guard: padded/masked entries can only appear if fewer than 50 real
    # candidates remain, which cannot happen (C*T*8 >> 53)
    lse3 = _logsumexp(np.concatenate([x_tops, x_sel], axis=1))
    online = lse3 - x_tops.mean(axis=1)

    dbg = globals().get("_DEBUG_COMPS")
    if dbg is not None:
        dbg["intra"] = intra.copy()
        dbg["cross"] = cross.copy()
        dbg["online"] = online.copy()

    total = 0.0
    for c in range(C):
        m = cams_h == c
        if m.any():
            total += intra[m].mean() + cross[m].mean() + online[m].mean()
    return np.float32(total)
